# revision 36
# baseline (speedup 1.0000x reference)
"""Trainium2 Bass kernel for nn_LossWithBeliveMaps.

loss = mean((prediction - belive_map)^2) where belive_map is the 9x9-kernel
convolution of keypoint scatter masks summed over S channels.

Strategy (8 cores, data-parallel over batch B=8, one image per core):
  - Host preprocesses *indices only*: dedup (s,y,x) keypoints, assign each
    (keypoint, kernel-row) pair to a (row-block, col-block) cell of the
    512x512 output, and compute window-row indices into a table that holds
    every possible 64-wide shifted placement of each kernel row.
  - Device builds the belief map with a scatter-as-matmul formulation:
      * x-placement: dma_gather of 64-wide shifted kernel-row windows
        (exact fp32 values, zero padding included in the table)
      * y-placement: one-hot matrices built on VectorE (iota vs y compare),
        used as stationary operands of TensorE matmuls accumulating into PSUM
  - MSE: per row-block, one 2MB DMA loads pred[s, rows, :] as [128 x (8*512)],
    VectorE subtracts bm (broadcast over s via stride-0 AP), ScalarE squares
    with accum_out -> per-partition partial sums.
  - Host sums the 8 cores' partials (the scalar "all-reduce") and divides.
"""

import sys

sys.path.insert(0, "/opt/trn_rl_repo")

import numpy as np

import concourse.bass as bass
import concourse.bacc as bacc
import concourse.mybir as mybir
import concourse.tile as tile
from concourse.tile import add_dep_helper
from concourse.bass_utils import run_bass_kernel_spmd

B, N, S, H, W = 8, 32, 8, 512, 512
KS = 9
R = KS // 2  # 4
NCORES = 8
RBS = 128  # row-block size (partitions)
NRB = H // RBS  # 4
CBW = 64  # col-block width
NCB = W // CBW  # 8
PADL = CBW - 1  # 63: zero padding each side of a kernel row
NSHIFT = CBW + KS - 1  # 72 possible window placements per kernel row
ZROW = KS * NSHIFT  # index of the all-zero window row
VTROWS = ZROW + 1  # 649
CAP = 128  # slots per cell instance (= matmul contraction size)

f32 = mybir.dt.float32
i16 = mybir.dt.int16

dt_i16 = np.int16


def _preprocess(target):
    """Index-only preprocessing.

    Returns (ylocs, vidx, cells, nrb_insts):
      ylocs: (NCORES, 128, C) float32 - row-within-block per slot, -1 for pad
      vidx:  (NCORES, 16, C*8) int16  - dma_gather index layout; global slot
             j = ci*128 + p is stored at [j % 16, j // 16]
      cells: list of (rb, cb, start, stop) per instance, rb-major order
      nrb_insts: instances per row-block
    """
    per_core = []
    for b in range(NCORES):
        # triples (s, y, x); dedup exact duplicates (set semantics)
        xs = np.asarray(target[b])[..., 0].reshape(N, S)
        ys = np.asarray(target[b])[..., 1].reshape(N, S)
        triples = set()
        for n in range(N):
            for s in range(S):
                triples.add((s, int(ys[n, s]), int(xs[n, s])))
        cells = {}
        for (_s, y, x) in triples:
            cb_set = set()
            for e in (x - R, x + R):
                cb = e // CBW
                if 0 <= cb < NCB:
                    cb_set.add(cb)
            for r in range(KS):
                row = y + r - R
                if not (0 <= row < H):
                    continue
                rb, m = row // RBS, row % RBS
                for cb in cb_set:
                    shift = PADL + R + cb * CBW - x  # in [0, NSHIFT)
                    cells.setdefault((rb, cb), []).append((m, r * NSHIFT + shift))
        per_core.append(cells)

    # uniform instance structure across cores
    cell_insts = {}
    for rb in range(NRB):
        for cb in range(NCB):
            mx = max(len(pc.get((rb, cb), ())) for pc in per_core)
            cell_insts[(rb, cb)] = max(1, -(-mx // CAP))

    cells = []
    for rb in range(NRB):
        for cb in range(NCB):
            n = cell_insts[(rb, cb)]
            for i in range(n):
                cells.append((rb, cb, i == 0, i == n - 1))
    C = len(cells)
    nrb_insts = [sum(1 for c in cells if c[0] == rb) for rb in range(NRB)]

    ylocs = np.full((NCORES, CAP, C), -1.0, dtype=np.float32)
    vlin = np.full((NCORES, C * CAP), ZROW, dtype=dt_i16)
    for b in range(NCORES):
        pc = per_core[b]
        inst_of_cell = {}
        for ci, (rb, cb, _st, _sp) in enumerate(cells):
            inst_of_cell.setdefault((rb, cb), []).append(ci)
        for key, slots in pc.items():
            targets = inst_of_cell[key]
            for j, (m, vrow) in enumerate(slots):
                ci = targets[j // CAP]
                p = j % CAP
                ylocs[b, p, ci] = float(m)
                vlin[b, ci * CAP + p] = vrow
    # wrap into dma_gather layout: idx j -> [j % 16, j // 16], and replicate
    # the 16-channel block across all 8 gpsimd cores (128 partitions total)
    vidx16 = vlin.reshape(NCORES, C * CAP // 16, 16).transpose(0, 2, 1)
    vidx = np.ascontiguousarray(np.tile(vidx16, (1, 8, 1)))
    return ylocs, vidx, cells, nrb_insts


def _build_nc(C, cells, nrb_insts, debug_bm=False):
    nc = bacc.Bacc(
        "TRN2", target_bir_lowering=False, debug=False, num_devices=NCORES
    )
    pred_ap = nc.dram_tensor("pred", [S, H, W], f32, kind="ExternalInput").ap()
    cst_ap = nc.dram_tensor("cst", [CAP, C + 128], f32, kind="ExternalInput").ap()
    vidx_ap = nc.dram_tensor("vidx", [128, C * 8], i16, kind="ExternalInput").ap()
    vtab_ap = nc.dram_tensor("vtab", [VTROWS, CBW], f32, kind="ExternalInput").ap()
    out_ap = nc.dram_tensor("out", [128, 2 * NRB + 2], f32, kind="ExternalOutput").ap()
    if debug_bm:
        bmout_ap = nc.dram_tensor("bm_out", [H, W], f32, kind="ExternalOutput").ap()
        vout_ap = nc.dram_tensor(
            "v_out", [CAP, C * CBW], f32, kind="ExternalOutput"
        ).ap()

    with tile.TileContext(nc) as tc:
        with (
            tc.tile_pool(name="const", bufs=1) as const_pool,
            tc.tile_pool(name="vwin", bufs=1) as v_pool,
            tc.tile_pool(name="onehot", bufs=4) as oh_pool,
            tc.tile_pool(name="psum", bufs=4, space="PSUM") as psum_pool,
            tc.tile_pool(name="bm", bufs=4) as bm_pool,
            tc.tile_pool(name="pred", bufs=6) as pred_pool,
        ):
            cst_sb = const_pool.tile([CAP, C + 128], f32)
            vidx_sb = const_pool.tile([128, C * 8], i16)
            acc = const_pool.tile([128, 2 * NRB + 2], f32)
            nc.sync.dma_start(out=vidx_sb[:], in_=vidx_ap[:])
            nc.sync.dma_start(out=cst_sb[:], in_=cst_ap[:])
            ylocs_sb = cst_sb
            iota_sb = cst_sb[:, C : C + 128]

            # phase 1: belief-map build. Two dma_gathers fetch every
            # x-placed kernel-row window; one-hot matmuls scatter them to
            # their output rows in PSUM. The first pred chunk is issued
            # with no ordering dep so the DMA engines stay busy during the
            # gathers' descriptor generation.
            vg = v_pool.tile([CAP, C * CBW], f32)
            half = (nrb_insts[0] + nrb_insts[1])  # instances of rb 0+1
            g1 = nc.gpsimd.dma_gather(
                vg[:, : half * CBW].rearrange("p (g e) -> p g e", e=CBW),
                vtab_ap[:],
                vidx_sb[:, : half * 8],
                half * CAP,
                half * CAP,
                CBW,
                single_packet=False,
            )
            g2 = nc.gpsimd.dma_gather(
                vg[:, half * CBW :].rearrange("p (g e) -> p g e", e=CBW),
                vtab_ap[:],
                vidx_sb[:, half * 8 :],
                (C - half) * CAP,
                (C - half) * CAP,
                CBW,
                single_packet=False,
            )
            bms = []
            i0 = 0
            for rb in range(NRB):
                n_inst = nrb_insts[rb]
                i1 = i0 + n_inst
                psum_rb = psum_pool.tile([128, W], f32, space="PSUM")
                for ci in range(i0, i1):
                    _rb, cb, start, stop = cells[ci]
                    assert _rb == rb
                    oh = oh_pool.tile([CAP, 128], f32)
                    nc.vector.tensor_scalar(
                        oh[:],
                        iota_sb,
                        ylocs_sb[:, ci : ci + 1],
                        None,
                        mybir.AluOpType.is_equal,
                    )
                    nc.tensor.matmul(
                        out=psum_rb[:, cb * CBW : (cb + 1) * CBW],
                        lhsT=oh[:],
                        rhs=vg[:, ci * CBW : (ci + 1) * CBW],
                        start=start,
                        stop=stop,
                    )
                bm_rb = bm_pool.tile([128, W], f32)
                nc.scalar.copy(out=bm_rb[:], in_=psum_rb[:])
                bms.append(bm_rb)
                if debug_bm:
                    nc.sync.dma_start(
                        out=bmout_ap[rb * RBS : (rb + 1) * RBS, :], in_=bm_rb[:]
                    )
                    nc.sync.dma_start(
                        out=vout_ap[:, i0 * CBW : i1 * CBW], in_=vg[:, i0 * CBW : i1 * CBW]
                    )
                i0 = i1

            # phase 2: stream pred (HWDGE) behind the gather and accumulate
            # the squared error. Finer chunks near the end shrink the
            # compute tail after the last DMA byte lands.
            pred_i = 0
            acc_col = 0
            for rb in range(NRB):
                bm_rb = bms[rb]
                nchunk = 4 if rb == NRB - 1 else 2
                sc = S // nchunk
                for c in range(nchunk):
                    pt = pred_pool.tile([128, sc, W], f32)
                    pdma = nc.sync.dma_start(
                        out=pt[:],
                        in_=pred_ap[
                            c * sc : (c + 1) * sc,
                            rb * RBS : (rb + 1) * RBS,
                            :,
                        ].rearrange("s p c -> p s c"),
                    )
                    if pred_i == 1:
                        # order the bulk pred stream behind the tiny
                        # critical-path gathers on the DMA engines; the
                        # first chunk runs free to fill the gather-DGE
                        # window
                        add_dep_helper(
                            pdma.ins, g1.ins, True, "pred waits on gather"
                        )
                    pred_i += 1
                    bm_b = bm_rb[:, None, :].to_broadcast([128, sc, W])
                    nc.vector.tensor_tensor(
                        out=pt[:], in0=pt[:], in1=bm_b, op=mybir.AluOpType.subtract
                    )
                    nc.scalar.activation(
                        out=pt[:],
                        in_=pt[:],
                        func=mybir.ActivationFunctionType.Square,
                        accum_out=acc[:, acc_col : acc_col + 1],
                    )
                    acc_col += 1

            nc.sync.dma_start(out=out_ap[:], in_=acc[:])

    nc.compile()
    return nc


_IOTA = np.tile(np.arange(128, dtype=np.float32), (128, 1))


def _make_vtab(gk):
    """All 64-wide shifted placements of each padded kernel row (+ zero row)."""
    vtab = np.zeros((VTROWS, CBW), dtype=np.float32)
    padded = np.zeros((KS, PADL + KS + PADL), dtype=np.float32)
    # conv_general_dilated is cross-correlation: a keypoint at (y, x)
    # stamps the FLIPPED kernel around itself
    padded[:, PADL : PADL + KS] = gk[::-1, ::-1]
    for r in range(KS):
        for s in range(NSHIFT):
            vtab[r * NSHIFT + s] = padded[r, s : s + CBW]
    return vtab


def kernel(prediction, target, gaussian_kernel):
    prediction = np.ascontiguousarray(np.asarray(prediction, dtype=np.float32))
    target = np.asarray(target, dtype=np.int32)
    gk = np.asarray(gaussian_kernel, dtype=np.float32)

    ylocs, vidx, cells, nrb_insts = _preprocess(target)
    C = len(cells)
    nc = _build_nc(C, cells, nrb_insts)
    vtab = _make_vtab(gk)

    in_maps = [
        {
            "pred": prediction[b],
            "cst": np.concatenate([ylocs[b], _IOTA], axis=1),
            "vidx": vidx[b],
            "vtab": vtab,
        }
        for b in range(NCORES)
    ]
    res = run_bass_kernel_spmd(nc, in_maps, list(range(NCORES)), trace=False)
    total = sum(np.sum(res.results[b]["out"], dtype=np.float64) for b in range(NCORES))
    return np.float32(total / (B * S * H * W))
